# revision 42
# baseline (speedup 1.0000x reference)
"""TRN2 Bass kernel for nn_Attention_30485677867708.

Computes, for input [N=8192, D=256] and weights Q,K,V_down [D,H=128], V_up [H,D]:
    q = input @ Q; k = input @ K; v = input @ V_down
    attn = softmax(q @ k.T, axis=1)
    out  = (attn @ v) @ V_up            -> [N, D] fp32

Row-sharded SPMD over 8 NeuronCores (1024 rows each); K/V path replicated.

Per-core pipeline:
  prep: keyT/qT computed in fp16 hi/lo pairs (3-term matmuls == fp32-grade
        scores), v in fp16, from host-supplied transposed fp16 hi/lo input.
  scores: S[rows,keys] 1024-key chunks via 3-term fp16 matmuls into PSUM.
  softmax: DVE chunk-max (negated) -> ACT exp(bias=-B_c, accum_out=sums_c)
           -> P fp16 in SBUF; per-row-tile epilogue folds exp(B_c - M)/rowsum
           into one per-chunk tensor_scalar rescale => P becomes attn weights.
  AV: P^T produced by DMA xbar transposes (SBUF->SBUF, batched 128x128
      blocks, two row-tiles packed side by side), then fp16 matmuls
      accumulate oT[H, rows] in two psum half-groups.
  out: dE[rows, D] = oT.T @ V_up in fp32, DMA out.
"""

import numpy as np
from contextlib import ExitStack

import concourse.bacc as bacc
from concourse import mybir
from concourse.tile import TileContext, add_dep_helper
from concourse.bass_utils import run_bass_kernel_spmd

f32 = mybir.dt.float32
f16 = mybir.dt.float16
EXP = mybir.ActivationFunctionType.Exp
MAX = mybir.AluOpType.max
AXX = mybir.AxisListType.X

N_CORES = 8


def build(N=8192, D=256, H=128, RPC=1024):
    """Build the per-core SPMD program. RPC = rows per core."""
    CHUNK = 1024                  # keys per softmax chunk (2 psum banks)
    NCH = N // CHUNK
    RT = RPC // 128               # row tiles per core
    GRP = 2                       # row tiles per AV group
    NG = RT // GRP
    NKT = N // 128                # key tiles
    HKT = NKT // 2                # key tiles per AV half
    KB = 512                      # matmul moving width
    NKB = N // KB

    nc = bacc.Bacc("TRN2", target_bir_lowering=False)

    # per-core input is host-rotated along keys so this core's own rows are
    # the first RPC columns (softmax over keys is permutation-invariant).
    inh = nc.dram_tensor("inh", [D, N], f16, kind="ExternalInput")
    inl = nc.dram_tensor("inl", [D, N], f16, kind="ExternalInput")
    # [Qh | Ql | Kh | Kl | Vdh] each [D, H]
    wpk = nc.dram_tensor("wpk", [D, 5 * H], f16, kind="ExternalInput")
    vup = nc.dram_tensor("vup", [H, D], f32, kind="ExternalInput")
    out = nc.dram_tensor("out", [RPC, D], f32, kind="ExternalOutput")

    # Matmuls that share a PSUM zero region (bank) across separate start/stop
    # sequences must not interleave on PE: chain those explicitly. Everything
    # else is left to the scheduler (accumulation RAW deps already order the
    # matmuls within one group).
    chain_last = [None]

    def mm(*args, chain=False, **kw):
        inst = nc.tensor.matmul(*args, **kw)
        if chain:
            if chain_last[0] is not None:
                add_dep_helper(
                    inst.ins, chain_last[0].ins, sync=False, reason="bank-order"
                )
            chain_last[0] = inst
        return inst

    with TileContext(nc) as tc, ExitStack() as ctx:
        wp = ctx.enter_context(tc.tile_pool(name="wp", bufs=1))
        big = ctx.enter_context(tc.tile_pool(name="big", bufs=1))

        wp0 = wp.tile([128, 5 * H], f16, tag="wp0")
        wp1 = wp.tile([128, 5 * H], f16, tag="wp1")
        vu = wp.tile([H, D], f32, tag="vu")
        nc.sync.dma_start(wp0[:], wpk[0:128, :])
        nc.sync.dma_start(wp1[:], wpk[128:256, :])
        nc.sync.dma_start(vu[:], vup[:])

        kh = big.tile([128, N], f16, tag="kh")
        kl = big.tile([128, N], f16, tag="kl")
        vsb = big.tile([128, N], f16, tag="vsb")
        qh = big.tile([128, RPC], f16, tag="qh")
        ql = big.tile([128, RPC], f16, tag="ql")

        # ---------------- prep ----------------
        with ExitStack() as prep:
            ipool = prep.enter_context(tc.tile_pool(name="ipool", bufs=1))
            pps = prep.enter_context(tc.tile_pool(name="pps", bufs=2, space="PSUM"))

            ih = [
                ipool.tile([128, N], f16, tag=f"ih{c}", name=f"ih{c}")
                for c in range(2)
            ]
            il = [
                ipool.tile([128, N], f16, tag=f"il{c}", name=f"il{c}")
                for c in range(2)
            ]
            DSL = 2048
            spans = [(0, 512), (512, 1024), (1024, 2048)] + [
                (j * DSL, (j + 1) * DSL) for j in range(1, N // DSL)
            ]
            for lo, hi in spans:
                js = slice(lo, hi)
                for c in range(2):
                    sl = slice(c * 128, (c + 1) * 128)
                    nc.sync.dma_start(ih[c][:, js], inh[sl, js])
                    nc.sync.dma_start(il[c][:, js], inl[sl, js])

            wslice = lambda c, i: (wp0 if c == 0 else wp1)[:, i * H : (i + 1) * H]

            def hilo3(ps_ap, w_hi_i, w_lo_i, mov_h, mov_l):
                for c in range(2):
                    mm(ps_ap, wslice(c, w_hi_i), mov_h[c], start=(c == 0), stop=False)
                    mm(ps_ap, wslice(c, w_hi_i), mov_l[c], start=False, stop=False)
                    mm(ps_ap, wslice(c, w_lo_i), mov_h[c], start=False,
                       stop=(c == 1))

            # qT -> qh/ql fp16 pair
            QB = min(KB, RPC)
            for b in range(RPC // QB):
                rs = slice(b * QB, (b + 1) * QB)
                pq = pps.tile([128, QB], f32, tag="pq")
                hilo3(pq[:], 0, 1, [t[:, rs] for t in ih], [t[:, rs] for t in il])
                nc.scalar.copy(qh[:, rs], pq[:])
                nc.vector.tensor_sub(ql[:, rs], pq[:], qh[:, rs])

            # keyT -> kh/kl fp16 pair; v via vT blocks + xbar transpose
            for b in range(NKB):
                ks = slice(b * KB, (b + 1) * KB)
                pk = pps.tile([128, KB], f32, tag="pk")
                hilo3(pk[:], 2, 3, [t[:, ks] for t in ih], [t[:, ks] for t in il])
                nc.scalar.copy(kh[:, ks], pk[:])
                nc.vector.tensor_sub(kl[:, ks], pk[:], kh[:, ks])
                pv = pps.tile([128, KB], f32, tag="pv")
                mm(pv[:], wslice(0, 4), ih[0][:, ks], start=True, stop=False)
                mm(pv[:], wslice(1, 4), ih[1][:, ks], start=False, stop=True)
                vts = ipool.tile([128, KB], f16, tag="vts", bufs=3, name=f"vts{b}")
                nc.scalar.copy(vts[:], pv[:])
                nc.sync.dma_start(
                    vsb[:].rearrange("p (a b) -> p a b", b=128)[:, 4 * b : 4 * b + 4, :],
                    vts[:],
                    transpose=True,
                )

        # ---------------- main ----------------
        # PSUM budget (8 banks): spsum 3x2 + opsum 1 + dpsum 1
        ppool = ctx.enter_context(tc.tile_pool(name="ppool", bufs=2 * GRP))
        smalls = ctx.enter_context(tc.tile_pool(name="smalls", bufs=4))
        ptsb = ctx.enter_context(tc.tile_pool(name="ptsb", bufs=4))
        ostr = ctx.enter_context(tc.tile_pool(name="ostr", bufs=3))
        spsum = ctx.enter_context(tc.tile_pool(name="spsum", bufs=3, space="PSUM"))
        opsum = ctx.enter_context(tc.tile_pool(name="opsum", bufs=1, space="PSUM"))
        dpsum = ctx.enter_context(tc.tile_pool(name="dpsum", bufs=1, space="PSUM"))

        P_tiles = {}
        pts_tiles = {}

        def softmax_tile(rt, pre_chunk=None):
            P = ppool.tile([128, N], f16, tag="P")
            P_tiles[rt] = P
            negB = smalls.tile([128, NCH], f32, tag="negB")
            sums = smalls.tile([128, NCH], f32, tag="sums")
            lh = qh[:, rt * 128 : (rt + 1) * 128]
            ll = ql[:, rt * 128 : (rt + 1) * 128]
            for c in range(NCH):
                if pre_chunk is not None:
                    pre_chunk(c)
                ps = spsum.tile([128, CHUNK], f32, tag="ps")
                for hblk in range(CHUNK // KB):
                    o = ps[:, hblk * KB : (hblk + 1) * KB]
                    ks = slice(c * CHUNK + hblk * KB, c * CHUNK + (hblk + 1) * KB)
                    mm(o, lh, kh[:, ks], start=True, stop=False)
                    mm(o, lh, kl[:, ks], start=False, stop=False)
                    mm(o, ll, kh[:, ks], start=False, stop=True)
                nc.vector.tensor_reduce(
                    negB[:, c : c + 1], ps[:], axis=AXX, op=MAX, negate=True
                )
                nc.scalar.activation(
                    P[:, c * CHUNK : (c + 1) * CHUNK],
                    ps[:],
                    EXP,
                    bias=negB[:, c : c + 1],
                    scale=1.0,
                    accum_out=sums[:, c : c + 1],
                )
            negM = smalls.tile([128, 1], f32, tag="negM")
            nc.vector.tensor_reduce(
                negM[:], negB[:], axis=AXX, op=mybir.AluOpType.min
            )
            F = smalls.tile([128, NCH], f32, tag="F")
            nc.scalar.activation(F[:], negB[:], EXP, bias=negM[:], scale=-1.0)
            T = smalls.tile([128, NCH], f32, tag="T")
            nc.vector.tensor_mul(T[:], F[:], sums[:])
            S = smalls.tile([128, 1], f32, tag="S")
            nc.vector.tensor_reduce(S[:], T[:], axis=AXX, op=mybir.AluOpType.add)
            R = smalls.tile([128, 1], f32, tag="R")
            nc.vector.reciprocal(R[:], S[:])
            G = smalls.tile([128, NCH], f32, tag="G")
            nc.vector.tensor_scalar_mul(G[:], F[:], R[:])
            for c in range(NCH):
                sl = slice(c * CHUNK, (c + 1) * CHUNK)
                if c % 4 == 3:
                    # offload a share of the rescales to ACT (affine copy)
                    nc.scalar.activation(
                        P[:, sl],
                        P[:, sl],
                        mybir.ActivationFunctionType.Copy,
                        scale=G[:, c : c + 1],
                    )
                else:
                    nc.vector.tensor_scalar_mul(P[:, sl], P[:, sl], G[:, c : c + 1])

        def transpose_rt(rt):
            """DMA xbar transposes for one row tile into its group's half
            tiles [128, HKT, GRP*128]; issued right after the rescale so the
            DMA overlaps the next tile's softmax."""
            g, j = rt // GRP, rt % GRP
            for h in range(2):
                if j == 0:
                    # [part, rt-in-group, key-tile, row] -- per-rt contiguous so
                    # the xbar M2S writes concatenate into large packets
                    pts_tiles[(g, h)] = ptsb.tile(
                        [128, GRP, HKT, 128], f16, tag="pts", name=f"pts_{g}_{h}"
                    )
                pts = pts_tiles[(g, h)]
                # split across DMA queues so the xbar transposes keep up
                NSP = 2
                SPK = HKT // NSP  # key tiles per sub-dma
                for sp in range(NSP):
                    nc.sync.dma_start(
                        pts[:][:, j, sp * SPK : (sp + 1) * SPK, :],
                        P_tiles[rt][
                            :,
                            h * (N // 2) + sp * SPK * 128 : h * (N // 2)
                            + (sp + 1) * SPK * 128,
                        ],
                        transpose=True,
                    )

        oTs_tiles = {}

        def av_group(g):
            oab = opsum.tile([128, 2 * GRP * 128], f32, tag="oab")
            for h in range(2):
                oacc = oab[:, h * GRP * 128 : (h + 1) * GRP * 128]
                pts = pts_tiles.pop((g, h))
                for i in range(HKT):
                    kt = h * HKT + i
                    mm(
                        oacc,
                        vsb[:, kt * 128 : (kt + 1) * 128],
                        pts[:][:, :, i, :],
                        start=(i == 0),
                        stop=(i == HKT - 1),
                        chain=True,
                    )
            oTs = ostr.tile([128, GRP * 128], f32, tag="oTs")
            nc.scalar.copy(oTs[:], oab[:, 0 : GRP * 128])
            nc.vector.tensor_add(
                oTs[:], oTs[:], oab[:, GRP * 128 : 2 * GRP * 128]
            )
            oTs_tiles[g] = oTs

        def de_group(g):
            # emitted late so the pd matmuls never stall the fenced PE stream
            oTs = oTs_tiles.pop(g)
            for j in range(GRP):
                rt = g * GRP + j
                pd = dpsum.tile([128, D], f32, tag="pd")
                mm(pd[:], oTs[:, j * 128 : (j + 1) * 128], vu[:], start=True, stop=True)
                dEs = ostr.tile([128, D], f32, tag="dEs")
                nc.scalar.copy(dEs[:], pd[:])
                nc.sync.dma_start(out[rt * 128 : (rt + 1) * 128, :], dEs[:])

        for rt in range(RT):
            softmax_tile(rt)
            transpose_rt(rt)
            if rt % GRP == GRP - 1:
                g = rt // GRP
                if g >= 1:
                    av_group(g - 1)
                if g >= 2:
                    de_group(g - 2)
        av_group(NG - 1)
        for g in sorted(oTs_tiles.keys()):
            de_group(g)

    return nc


def _split16(x):
    hi = x.astype(np.float16)
    lo = (x - hi.astype(np.float32)).astype(np.float16)
    return hi, lo


def kernel(input, Q, K, V_down, V_up):
    input = np.asarray(input, np.float32)
    Q = np.asarray(Q, np.float32)
    K = np.asarray(K, np.float32)
    V_down = np.asarray(V_down, np.float32)
    V_up = np.asarray(V_up, np.float32)

    N, D = input.shape
    H = Q.shape[1]
    RPC = N // N_CORES

    inT = np.ascontiguousarray(input.T)  # [D, N]
    inh, inl = _split16(inT)
    Qh, Ql = _split16(Q)
    Kh, Kl = _split16(K)
    Vdh = V_down.astype(np.float16)
    wpk = np.ascontiguousarray(np.concatenate([Qh, Ql, Kh, Kl, Vdh], axis=1))

    nc = build(N=N, D=D, H=H, RPC=RPC)
    nc.finalize()

    in_maps = []
    for c in range(N_CORES):
        r = c * RPC
        in_maps.append(
            {
                "inh": np.ascontiguousarray(np.roll(inh, -r, axis=1)),
                "inl": np.ascontiguousarray(np.roll(inl, -r, axis=1)),
                "wpk": wpk,
                "vup": V_up,
            }
        )

    res = run_bass_kernel_spmd(nc, in_maps, core_ids=list(range(N_CORES)))
    return np.concatenate([res.results[c]["out"] for c in range(N_CORES)], axis=0)


# revision 43
# speedup vs baseline: 1.0032x; 1.0032x over previous
"""TRN2 Bass kernel for nn_Attention_30485677867708.

Computes, for input [N=8192, D=256] and weights Q,K,V_down [D,H=128], V_up [H,D]:
    q = input @ Q; k = input @ K; v = input @ V_down
    attn = softmax(q @ k.T, axis=1)
    out  = (attn @ v) @ V_up            -> [N, D] fp32

Row-sharded SPMD over 8 NeuronCores (1024 rows each); K/V path replicated.

Per-core pipeline:
  prep: keyT/qT computed in fp16 hi/lo pairs (3-term matmuls == fp32-grade
        scores), v in fp16, from host-supplied transposed fp16 hi/lo input.
  scores: S[rows,keys] 1024-key chunks via 3-term fp16 matmuls into PSUM.
  softmax: DVE chunk-max (negated) -> ACT exp(bias=-B_c, accum_out=sums_c)
           -> P fp16 in SBUF; per-row-tile epilogue folds exp(B_c - M)/rowsum
           into one per-chunk tensor_scalar rescale => P becomes attn weights.
  AV: P^T produced by DMA xbar transposes (SBUF->SBUF, batched 128x128
      blocks, two row-tiles packed side by side), then fp16 matmuls
      accumulate oT[H, rows] in two psum half-groups.
  out: dE[rows, D] = oT.T @ V_up in fp32, DMA out.
"""

import numpy as np
from contextlib import ExitStack

import concourse.bacc as bacc
from concourse import mybir
from concourse.tile import TileContext, add_dep_helper
from concourse.bass_utils import run_bass_kernel_spmd

f32 = mybir.dt.float32
f16 = mybir.dt.float16
EXP = mybir.ActivationFunctionType.Exp
MAX = mybir.AluOpType.max
AXX = mybir.AxisListType.X

N_CORES = 8


def build(N=8192, D=256, H=128, RPC=1024):
    """Build the per-core SPMD program. RPC = rows per core."""
    CHUNK = 1024                  # keys per softmax chunk (2 psum banks)
    NCH = N // CHUNK
    RT = RPC // 128               # row tiles per core
    GRP = 2                       # row tiles per AV group
    NG = RT // GRP
    NKT = N // 128                # key tiles
    HKT = NKT // 2                # key tiles per AV half
    KB = 512                      # matmul moving width
    NKB = N // KB

    nc = bacc.Bacc("TRN2", target_bir_lowering=False)

    # per-core input is host-rotated along keys so this core's own rows are
    # the first RPC columns (softmax over keys is permutation-invariant).
    inh = nc.dram_tensor("inh", [D, N], f16, kind="ExternalInput")
    inl = nc.dram_tensor("inl", [D, N], f16, kind="ExternalInput")
    # [Qh | Ql | Kh | Kl | Vdh] each [D, H]
    wpk = nc.dram_tensor("wpk", [D, 5 * H], f16, kind="ExternalInput")
    vup = nc.dram_tensor("vup", [H, D], f32, kind="ExternalInput")
    out = nc.dram_tensor("out", [RPC, D], f32, kind="ExternalOutput")

    # Matmuls that share a PSUM zero region (bank) across separate start/stop
    # sequences must not interleave on PE: chain those explicitly. Everything
    # else is left to the scheduler (accumulation RAW deps already order the
    # matmuls within one group).
    chain_last = [None]

    def mm(*args, chain=False, **kw):
        inst = nc.tensor.matmul(*args, **kw)
        if chain:
            if chain_last[0] is not None:
                add_dep_helper(
                    inst.ins, chain_last[0].ins, sync=False, reason="bank-order"
                )
            chain_last[0] = inst
        return inst

    with TileContext(nc) as tc, ExitStack() as ctx:
        wp = ctx.enter_context(tc.tile_pool(name="wp", bufs=1))
        big = ctx.enter_context(tc.tile_pool(name="big", bufs=1))

        wp0 = wp.tile([128, 5 * H], f16, tag="wp0")
        wp1 = wp.tile([128, 5 * H], f16, tag="wp1")
        vu = wp.tile([H, D], f32, tag="vu")
        nc.sync.dma_start(wp0[:], wpk[0:128, :])
        nc.sync.dma_start(wp1[:], wpk[128:256, :])
        nc.sync.dma_start(vu[:], vup[:])

        kh = big.tile([128, N], f16, tag="kh")
        kl = big.tile([128, N], f16, tag="kl")
        vsb = big.tile([128, N], f16, tag="vsb")
        qh = big.tile([128, RPC], f16, tag="qh")
        ql = big.tile([128, RPC], f16, tag="ql")

        # ---------------- prep ----------------
        with ExitStack() as prep:
            ipool = prep.enter_context(tc.tile_pool(name="ipool", bufs=1))
            pps = prep.enter_context(tc.tile_pool(name="pps", bufs=2, space="PSUM"))

            ih = [
                ipool.tile([128, N], f16, tag=f"ih{c}", name=f"ih{c}")
                for c in range(2)
            ]
            il = [
                ipool.tile([128, N], f16, tag=f"il{c}", name=f"il{c}")
                for c in range(2)
            ]
            DSL = 2048
            spans = [(0, 512), (512, 1024), (1024, 2048)] + [
                (j * DSL, (j + 1) * DSL) for j in range(1, N // DSL)
            ]
            for lo, hi in spans:
                js = slice(lo, hi)
                for c in range(2):
                    sl = slice(c * 128, (c + 1) * 128)
                    nc.sync.dma_start(ih[c][:, js], inh[sl, js])
                    nc.sync.dma_start(il[c][:, js], inl[sl, js])

            wslice = lambda c, i: (wp0 if c == 0 else wp1)[:, i * H : (i + 1) * H]

            def hilo3(ps_ap, w_hi_i, w_lo_i, mov_h, mov_l):
                for c in range(2):
                    mm(ps_ap, wslice(c, w_hi_i), mov_h[c], start=(c == 0), stop=False)
                    mm(ps_ap, wslice(c, w_hi_i), mov_l[c], start=False, stop=False)
                    mm(ps_ap, wslice(c, w_lo_i), mov_h[c], start=False,
                       stop=(c == 1))

            # qT -> qh/ql fp16 pair
            QB = min(KB, RPC)
            for b in range(RPC // QB):
                rs = slice(b * QB, (b + 1) * QB)
                pq = pps.tile([128, QB], f32, tag="pq")
                hilo3(pq[:], 0, 1, [t[:, rs] for t in ih], [t[:, rs] for t in il])
                nc.scalar.copy(qh[:, rs], pq[:])
                nc.vector.tensor_sub(ql[:, rs], pq[:], qh[:, rs])

            # keyT -> kh/kl fp16 pair; v via vT blocks + xbar transpose
            for b in range(NKB):
                ks = slice(b * KB, (b + 1) * KB)
                pk = pps.tile([128, KB], f32, tag="pk")
                hilo3(pk[:], 2, 3, [t[:, ks] for t in ih], [t[:, ks] for t in il])
                nc.scalar.copy(kh[:, ks], pk[:])
                nc.vector.tensor_sub(kl[:, ks], pk[:], kh[:, ks])
                pv = pps.tile([128, KB], f32, tag="pv")
                mm(pv[:], wslice(0, 4), ih[0][:, ks], start=True, stop=False)
                mm(pv[:], wslice(1, 4), ih[1][:, ks], start=False, stop=True)
                vts = ipool.tile([128, KB], f16, tag="vts", bufs=3, name=f"vts{b}")
                nc.scalar.copy(vts[:], pv[:])
                nc.sync.dma_start(
                    vsb[:].rearrange("p (a b) -> p a b", b=128)[:, 4 * b : 4 * b + 4, :],
                    vts[:],
                    transpose=True,
                )

        # ---------------- main ----------------
        # PSUM budget (8 banks): spsum 3x2 + opsum 1 + dpsum 1
        ppool = ctx.enter_context(tc.tile_pool(name="ppool", bufs=2 * GRP))
        smalls = ctx.enter_context(tc.tile_pool(name="smalls", bufs=4))
        ptsb = ctx.enter_context(tc.tile_pool(name="ptsb", bufs=4))
        ostr = ctx.enter_context(tc.tile_pool(name="ostr", bufs=3))
        spsum = ctx.enter_context(tc.tile_pool(name="spsum", bufs=3, space="PSUM"))
        opsum = ctx.enter_context(tc.tile_pool(name="opsum", bufs=1, space="PSUM"))
        dpsum = ctx.enter_context(tc.tile_pool(name="dpsum", bufs=1, space="PSUM"))

        P_tiles = {}
        pts_tiles = {}

        def softmax_tile(rt, pre_chunk=None):
            P = ppool.tile([128, N], f16, tag="P")
            P_tiles[rt] = P
            negB = smalls.tile([128, NCH], f32, tag="negB")
            sums = smalls.tile([128, NCH], f32, tag="sums")
            lh = qh[:, rt * 128 : (rt + 1) * 128]
            ll = ql[:, rt * 128 : (rt + 1) * 128]
            for c in range(NCH):
                if pre_chunk is not None:
                    pre_chunk(c)
                ps = spsum.tile([128, CHUNK], f32, tag="ps")
                for hblk in range(CHUNK // KB):
                    o = ps[:, hblk * KB : (hblk + 1) * KB]
                    ks = slice(c * CHUNK + hblk * KB, c * CHUNK + (hblk + 1) * KB)
                    mm(o, lh, kh[:, ks], start=True, stop=False)
                    mm(o, lh, kl[:, ks], start=False, stop=False)
                    mm(o, ll, kh[:, ks], start=False, stop=True)
                nc.vector.tensor_reduce(
                    negB[:, c : c + 1], ps[:], axis=AXX, op=MAX, negate=True
                )
                nc.scalar.activation(
                    P[:, c * CHUNK : (c + 1) * CHUNK],
                    ps[:],
                    EXP,
                    bias=negB[:, c : c + 1],
                    scale=1.0,
                    accum_out=sums[:, c : c + 1],
                )
            negM = smalls.tile([128, 1], f32, tag="negM")
            nc.vector.tensor_reduce(
                negM[:], negB[:], axis=AXX, op=mybir.AluOpType.min
            )
            F = smalls.tile([128, NCH], f32, tag="F")
            nc.scalar.activation(F[:], negB[:], EXP, bias=negM[:], scale=-1.0)
            T = smalls.tile([128, NCH], f32, tag="T")
            nc.vector.tensor_mul(T[:], F[:], sums[:])
            S = smalls.tile([128, 1], f32, tag="S")
            nc.vector.tensor_reduce(S[:], T[:], axis=AXX, op=mybir.AluOpType.add)
            R = smalls.tile([128, 1], f32, tag="R")
            nc.vector.reciprocal(R[:], S[:])
            G = smalls.tile([128, NCH], f32, tag="G")
            nc.vector.tensor_scalar_mul(G[:], F[:], R[:])
            for c in range(NCH):
                sl = slice(c * CHUNK, (c + 1) * CHUNK)
                if c % 4 == 3:
                    # offload a share of the rescales to ACT (affine copy)
                    nc.scalar.activation(
                        P[:, sl],
                        P[:, sl],
                        mybir.ActivationFunctionType.Copy,
                        scale=G[:, c : c + 1],
                    )
                else:
                    nc.vector.tensor_scalar_mul(P[:, sl], P[:, sl], G[:, c : c + 1])

        def transpose_rt(rt):
            """DMA xbar transposes for one row tile into its group's half
            tiles [128, HKT, GRP*128]; issued right after the rescale so the
            DMA overlaps the next tile's softmax."""
            g, j = rt // GRP, rt % GRP
            for h in range(2):
                if j == 0:
                    # [part, rt-in-group, key-tile, row] -- per-rt contiguous so
                    # the xbar M2S writes concatenate into large packets
                    pts_tiles[(g, h)] = ptsb.tile(
                        [128, GRP, HKT, 128], f16, tag="pts", name=f"pts_{g}_{h}"
                    )
                pts = pts_tiles[(g, h)]
                # split across DMA queues so the xbar transposes keep up
                NSP = 2
                SPK = HKT // NSP  # key tiles per sub-dma
                for sp in range(NSP):
                    nc.sync.dma_start(
                        pts[:][:, j, sp * SPK : (sp + 1) * SPK, :],
                        P_tiles[rt][
                            :,
                            h * (N // 2) + sp * SPK * 128 : h * (N // 2)
                            + (sp + 1) * SPK * 128,
                        ],
                        transpose=True,
                    )

        oTs_tiles = {}
        av_oabs = {}

        def av_half(g, h, oacc):
            pts = pts_tiles.pop((g, h))
            for i in range(HKT):
                kt = h * HKT + i
                mm(
                    oacc,
                    vsb[:, kt * 128 : (kt + 1) * 128],
                    pts[:][:, :, i, :],
                    start=(i == 0),
                    stop=(i == HKT - 1),
                    chain=True,
                )

        def av_start(g):
            oab = opsum.tile([128, 2 * GRP * 128], f32, tag="oab", name=f"oab{g}")
            av_oabs[g] = oab
            av_half(g, 0, oab[:, 0 : GRP * 128])

        def av_finish(g):
            oab = av_oabs.pop(g)
            av_half(g, 1, oab[:, GRP * 128 : 2 * GRP * 128])
            oTs = ostr.tile([128, GRP * 128], f32, tag="oTs")
            nc.scalar.copy(oTs[:], oab[:, 0 : GRP * 128])
            nc.vector.tensor_add(
                oTs[:], oTs[:], oab[:, GRP * 128 : 2 * GRP * 128]
            )
            oTs_tiles[g] = oTs

        def de_group(g):
            # emitted late so the pd matmuls never stall the fenced PE stream
            oTs = oTs_tiles.pop(g)
            for j in range(GRP):
                rt = g * GRP + j
                pd = dpsum.tile([128, D], f32, tag="pd")
                mm(pd[:], oTs[:, j * 128 : (j + 1) * 128], vu[:], start=True, stop=True)
                dEs = ostr.tile([128, D], f32, tag="dEs")
                nc.scalar.copy(dEs[:], pd[:])
                nc.sync.dma_start(out[rt * 128 : (rt + 1) * 128, :], dEs[:])

        for rt in range(RT):
            softmax_tile(rt)
            transpose_rt(rt)
            if rt % GRP == 0 and rt // GRP >= 1:
                av_start(rt // GRP - 1)
            if rt % GRP == GRP - 1:
                g = rt // GRP
                if g >= 1:
                    av_finish(g - 1)
                if g >= 2:
                    de_group(g - 2)
        av_start(NG - 1)
        av_finish(NG - 1)
        for g in sorted(oTs_tiles.keys()):
            de_group(g)

    return nc


def _split16(x):
    hi = x.astype(np.float16)
    lo = (x - hi.astype(np.float32)).astype(np.float16)
    return hi, lo


def kernel(input, Q, K, V_down, V_up):
    input = np.asarray(input, np.float32)
    Q = np.asarray(Q, np.float32)
    K = np.asarray(K, np.float32)
    V_down = np.asarray(V_down, np.float32)
    V_up = np.asarray(V_up, np.float32)

    N, D = input.shape
    H = Q.shape[1]
    RPC = N // N_CORES

    inT = np.ascontiguousarray(input.T)  # [D, N]
    inh, inl = _split16(inT)
    Qh, Ql = _split16(Q)
    Kh, Kl = _split16(K)
    Vdh = V_down.astype(np.float16)
    wpk = np.ascontiguousarray(np.concatenate([Qh, Ql, Kh, Kl, Vdh], axis=1))

    nc = build(N=N, D=D, H=H, RPC=RPC)
    nc.finalize()

    in_maps = []
    for c in range(N_CORES):
        r = c * RPC
        in_maps.append(
            {
                "inh": np.ascontiguousarray(np.roll(inh, -r, axis=1)),
                "inl": np.ascontiguousarray(np.roll(inl, -r, axis=1)),
                "wpk": wpk,
                "vup": V_up,
            }
        )

    res = run_bass_kernel_spmd(nc, in_maps, core_ids=list(range(N_CORES)))
    return np.concatenate([res.results[c]["out"] for c in range(N_CORES)], axis=0)
